# revision 25
# baseline (speedup 1.0000x reference)
"""GCN (3-layer) Bass kernel for 8 TRN2 NeuronCores, SPMD.

Math: out = A_hat @ relu(A_hat @ relu(A_hat @ X W1 + b1) W2 + b2) W3 + b3
A_hat = D^-1/2 (A + I) D^-1/2.

Key design (v2 — DVE-aggregated, lane-slotted gather):
  The PE-throttled bottleneck of v1 (one one-hot matmul per 128 messages)
  is gone: messages are gathered DIRECTLY INTO their destination lane via
  a host-computed permutation, so the segment sum is a plain DVE add of
  [128 lanes, tiles, 128 feat] blocks.  Per layer the PE only does the
  dense GEMM (49 transposes + 49 matmuls).

  - Nodes are permuted host-side: sorted by degree into 49 "bands"; band
    t supplies tile t of every core (128 lanes/core).  Degree-banding
    equalizes per-lane message counts, so the rectangular (lane x level)
    slot grid wastes little.
  - Each node is assigned a half (lane<64 -> table 0) by a greedy that
    balances, for every destination, its in-edges across halves (keeps
    per-(half,lane) slot counts near deg/2).  Table h is AllGathered from
    h_sent[lanes 64h:64h+64, :, :] — a rectangular partition-range DMA.
  - Slot stream per (half): for level k, for tiles t with K[h,t]>k, 128
    slots (lane-major).  Level-k slots for a run of tiles are contiguous,
    so one DVE tensor op accumulates a whole level: acc += mt_slice.
  - Empty slots gather a zeroed pad row of the table.
  - Self loops ride in the stream as ordinary slots (source = own row).
  - Bias enters as acc += crep (crep[d,t,f] = sqrt(deg) * b[f], host
    const), then the ACT epilogue computes relu(dinv^2 * acc) =
    a*relu(a*agg + b) = next layer's pre-scaled x.  Final layer:
    out = dinv * acc (ACT Copy).
"""

from contextlib import ExitStack

import numpy as np
import ml_dtypes

import concourse.bacc as bacc
import concourse.bass as bass
import concourse.mybir as mybir
from concourse.tile import TileContext
from concourse import library_config

BF16 = mybir.dt.bfloat16
F32 = mybir.dt.float32
I16 = mybir.dt.int16
P = 128
HL = 64          # lanes per half
TG = 4           # tiles per PSUM bank group (GEMM)


def preprocess(edge_index, n_nodes, n_cores=8, call_size=4096, seed=0):
    """Host-side graph preprocessing.

    Returns (sched, per_core_data, perm_info).
    perm_info: node -> (core, l) permutation plus inverse for unsharding.
    """
    src = np.asarray(edge_index[0], dtype=np.int64)
    dst = np.asarray(edge_index[1], dtype=np.int64)
    deg = (np.bincount(dst, minlength=n_nodes) + 1).astype(np.float32)
    dinv = (1.0 / np.sqrt(deg)).astype(np.float32)
    sqd = np.sqrt(deg).astype(np.float32)

    NT = (n_nodes + n_cores * P - 1) // (n_cores * P)
    S = NT * P                      # padded nodes per core
    NP = S * n_cores                # padded total
    n_dummy = NP - n_nodes

    # --- node permutation: degree-sorted bands; band t -> tile t ---
    order = np.argsort(-deg, kind="stable")        # real nodes, deg desc
    # node id -> (band, core, slot-in-(core,band))
    core_of = np.empty(NP, dtype=np.int64)
    band_of = np.empty(NP, dtype=np.int64)
    ranked = np.concatenate([order, np.arange(n_nodes, NP)])  # dummies last
    r = np.arange(NP)
    band_of[ranked] = r // (n_cores * P)
    core_of[ranked] = (r % (n_cores * P)) % n_cores

    # --- half balancing (greedy): exactly HL of each (core, band)'s 128
    # nodes to half 0, minimizing per-dst |c0-c1| ---
    # CSR by source over real edges
    e_order = np.argsort(src, kind="stable")
    s_sorted = src[e_order]
    d_sorted = dst[e_order]
    sptr = np.searchsorted(s_sorted, np.arange(n_nodes + 1))
    D = np.zeros(n_nodes, dtype=np.int32)          # c0 - c1 per dst
    quota = np.full((n_cores, NT, 2), HL, dtype=np.int32)
    half_of = np.empty(NP, dtype=np.int8)
    # process high out-degree first (they matter most)
    out_deg = sptr[1:] - sptr[:-1]
    proc = np.argsort(-out_deg, kind="stable")
    for u in proc:
        c, b = core_of[u], band_of[u]
        q0, q1 = quota[c, b, 0], quota[c, b, 1]
        if q0 == 0:
            h = 1
        elif q1 == 0:
            h = 0
        else:
            vs = d_sorted[sptr[u]:sptr[u + 1]]
            s = int(D[vs].sum()) if len(vs) else 0
            h = 1 if s > 0 else 0
        half_of[u] = h
        quota[c, b, h] -= 1
        if sptr[u + 1] > sptr[u]:
            vs = d_sorted[sptr[u]:sptr[u + 1]]
            np.add.at(D, vs, 1 - 2 * h)
    for u in range(n_nodes, NP):                   # dummies fill quotas
        c, b = core_of[u], band_of[u]
        h = 0 if quota[c, b, 0] > 0 else 1
        half_of[u] = h
        quota[c, b, h] -= 1
    assert (quota == 0).all()

    # --- quota-preserving swap refinement (minimize sum D^2 over dsts),
    # with incremental sD maintenance so decisions never go stale ---
    ed_order = np.argsort(d_sorted, kind="stable")   # edges sorted by dst
    in_src = s_sorted[ed_order]
    in_d = d_sorted[ed_order]
    in_ptr = np.searchsorted(in_d, np.arange(n_nodes + 1))
    from scipy.sparse import coo_matrix as _coo
    Adj = _coo((np.ones(len(src), np.int64), (src, dst)),
               shape=(n_nodes, n_nodes)).tocsr()
    sD = np.zeros(NP, np.int64)
    sD[:n_nodes] = Adj @ D.astype(np.int64)
    k_out = np.zeros(NP, np.int64)
    k_out[:n_nodes] = out_deg
    grp_id = core_of * NT + band_of                # node -> group
    grp_members = [np.nonzero(grp_id == g)[0] for g in range(n_cores * NT)]

    def _apply_flip(u, s):
        """Flip node u's half; D at its dsts changes by s (+-2)."""
        if u >= n_nodes:
            return
        vs = d_sorted[sptr[u]:sptr[u + 1]]
        D[vs] += s
        ins = np.concatenate([in_src[in_ptr[v]:in_ptr[v + 1]] for v in vs]) \
            if len(vs) else np.empty(0, np.int64)
        if len(ins):
            np.add.at(sD, ins, s)

    for _ in range(4):
        flipped = 0
        for g in range(n_cores * NT):
            members = grp_members[g]
            m0 = members[half_of[members] == 0]
            m1 = members[half_of[members] == 1]
            gain01 = sD[m0] - k_out[m0]            # flip 0->1 gain (x4)
            gain10 = -sD[m1] - k_out[m1]           # flip 1->0 gain
            c0 = m0[np.argsort(-gain01, kind="stable")]
            c1 = m1[np.argsort(-gain10, kind="stable")]
            g0 = np.sort(gain01)[::-1]
            g1 = np.sort(gain10)[::-1]
            for u0, u1, a, b in zip(c0, c1, g0, g1):
                if a + b <= 0:
                    break
                half_of[u0] = 1
                half_of[u1] = 0
                _apply_flip(u0, -2)
                _apply_flip(u1, 2)
                flipped += 2
        if flipped == 0:
            break

    # --- lane assignment: half0 -> lanes 0..63, half1 -> 64..127 ---
    lane_of = np.empty(NP, dtype=np.int64)
    nodes_by_cb = [[[] for _ in range(NT)] for _ in range(n_cores)]
    for u in ranked:                               # rank order within groups
        nodes_by_cb[core_of[u]][band_of[u]].append(u)
    for c in range(n_cores):
        for b in range(NT):
            grp = nodes_by_cb[c][b]
            assert len(grp) == P
            i0 = i1 = 0
            for u in grp:
                if half_of[u] == 0:
                    lane_of[u] = i0
                    i0 += 1
                else:
                    lane_of[u] = HL + i1
                    i1 += 1
            assert i0 == HL and i1 == HL

    l_of = band_of * P + lane_of                   # local row index
    # inverse permutation for output unshard: out row of node u
    perm_rows = core_of * S + l_of                 # node -> global padded row

    # --- table row index of each node (as source) ---
    RS = NT * HL + HL                              # shard rows per half (+pad)
    ZROW = NT * HL                                 # zero rows at shard tail
    tbl_row = core_of * RS + band_of * HL + (lane_of % HL)

    # --- per-destination source lists (table rows), split by src half ---
    # (self loops are added on-device as a direct DVE add of h_sent)
    msrc = src
    mdst = dst
    mh = half_of[msrc].astype(np.int64)
    mrow = tbl_row[msrc]
    # group by (dst, half): sort by (dst, half, mrow)
    key = (mdst * 2 + mh) * (RS * n_cores + 1) + mrow
    g_order = np.argsort(key, kind="stable")
    gd = mdst[g_order]
    gh = mh[g_order]
    grow = mrow[g_order]
    # counts per (dst, half)
    cnt = np.zeros((NP, 2), dtype=np.int64)
    np.add.at(cnt, (gd, gh), 1)
    # start offset of each (dst, half) run in the sorted stream
    run_len = cnt.reshape(-1)
    run_start = np.concatenate([[0], np.cumsum(run_len)[:-1]])

    # --- K levels per (half, tile): max over cores & lanes, monotone env ---
    K = np.zeros((2, NT), dtype=np.int64)
    cview = cnt.reshape(NP, 2)
    for h in range(2):
        per_node = cview[:, h]
        # per (band) max over all nodes in that band (all cores)
        Kt = np.zeros(NT, dtype=np.int64)
        np.maximum.at(Kt, band_of, per_node)
        K[h] = Kt
    for h in range(2):                             # monotone non-increasing
        for t in range(NT - 2, -1, -1):
            K[h, t] = max(K[h, t], K[h, t + 1])

    # level prefix sizes T[h][k] = #tiles with K[h,t] > k
    T_lvl = [[int((K[h] > k).sum()) for k in range(int(K[h].max()))]
             for h in range(2)]

    # --- build per-core idx streams (k-level-major) ---
    ZIDX = ZROW                                    # zero row of core 0
    node_at = np.empty(NP, dtype=np.int64)         # (c*S + l) -> node id
    node_at[perm_rows] = np.arange(NP)
    L0 = sum(T_lvl[0]) * P
    L1 = sum(T_lvl[1]) * P
    LT = L0 + L1
    per_core = []
    for c in range(n_cores):
        idx_stream = np.full(LT, ZIDX, dtype=np.int64)
        pos = 0
        for h in range(2):
            for k, Tk in enumerate(T_lvl[h]):
                nodes = node_at[c * S + np.arange(Tk * P)]   # (t,d) row-major
                starts = run_start[nodes * 2 + h]
                lens = run_len[nodes * 2 + h]
                sel = lens > k
                idx_stream[pos + np.nonzero(sel)[0]] = grow[starts[sel] + k]
                pos += Tk * P
        assert pos == LT
        idxw = idx_stream.astype(np.int16).reshape(LT // 16, 16).T
        idxw = np.tile(idxw, (8, 1))
        per_core.append(dict(idxw=np.ascontiguousarray(idxw)))

    # --- per-core constants (lane-major) ---
    ids = np.arange(S)
    for c in range(n_cores):
        nods = node_at[c * S + ids]
        dvc = np.zeros((P, NT), dtype=np.float32)
        dv2 = np.zeros((P, NT), dtype=np.float32)
        sq = np.zeros((P, NT), dtype=np.float32)
        real = nods < n_nodes
        dvc[ids[real] % P, ids[real] // P] = dinv[nods[real]]
        dv2[ids[real] % P, ids[real] // P] = dinv[nods[real]] ** 2
        sq[ids[real] % P, ids[real] // P] = sqd[nods[real]]
        per_core[c].update(dinvc=dvc, dinv2c=dv2, sqdc=sq)

    # --- gather calls ---
    calls = []   # (half, start, n)
    for h, (lo, ln) in enumerate(((0, L0), (L0, L1))):
        off = 0
        while off < ln:
            n = min(call_size, ln - off)
            calls.append((h, lo + off, n))
            off += n

    # --- DVE add segments: (is_init, tile0, ntiles, start_slot) ---
    # is_init: level (h=0, k=0) covers every tile and initializes acc.
    segs = []
    pos = 0
    for h in range(2):
        half_lo = 0 if h == 0 else L0
        for k, Tk in enumerate(T_lvl[h]):
            # split [0, Tk) tile range at call boundaries (grid per half)
            t0 = 0
            while t0 < Tk:
                rel = pos + t0 * P - half_lo
                room = (rel // call_size + 1) * call_size - rel
                t1 = min(Tk, t0 + room // P)
                segs.append((h == 0 and k == 0, t0, t1 - t0, pos + t0 * P))
                t0 = t1
            pos += Tk * P
    assert pos == LT
    assert T_lvl[0][0] == NT, "level (0,0) must cover all tiles"

    sched = dict(n_nodes=n_nodes, n_cores=n_cores, S=S, NT=NT, RS=RS,
                 K=K, T_lvl=T_lvl, L0=L0, L1=L1, LT=LT, calls=calls,
                 segs=segs, call_size=call_size)
    perm_info = dict(perm_rows=perm_rows, node_at=node_at, dinv=dinv,
                     sqd=sqd, n_dummy=n_dummy)
    return sched, per_core, perm_info


def build_nc(sched):
    """Build the SPMD Bass graph (identical for all 8 cores)."""
    S, NT, RS = sched["S"], sched["NT"], sched["RS"]
    calls, segs = sched["calls"], sched["segs"]
    n_cores = sched["n_cores"]
    call_size = sched["call_size"]
    LT = sched["LT"]
    TBL = RS * n_cores
    core_ids = list(range(n_cores))

    nc = bacc.Bacc("TRN2", target_bir_lowering=False, num_devices=n_cores,
                   num_swdge_queues=4)

    x_in = nc.dram_tensor("x", [S, P], F32, kind="ExternalInput")
    w_in = [nc.dram_tensor(f"w{i+1}", [P, P], BF16, kind="ExternalInput")
            for i in range(3)]
    crep_in = nc.dram_tensor("crep", [P, 3, NT * P], BF16,
                             kind="ExternalInput")
    dinv_in = nc.dram_tensor("dinvc", [P, NT], F32, kind="ExternalInput")
    dinv2_in = nc.dram_tensor("dinv2c", [P, NT], F32, kind="ExternalInput")
    ident_in = nc.dram_tensor("identb", [P, P], BF16, kind="ExternalInput")
    idxw_in = nc.dram_tensor("idxw", [P, LT // 16], I16, kind="ExternalInput")
    out_ext = nc.dram_tensor("out", [S, 64], F32, kind="ExternalOutput")

    with TileContext(nc) as tc, ExitStack() as ex:
        const = ex.enter_context(tc.tile_pool(name="const", bufs=1))
        dram = ex.enter_context(tc.tile_pool(name="dram", bufs=1, space="DRAM"))
        sb = ex.enter_context(tc.tile_pool(name="sb", bufs=2))
        msgp = ex.enter_context(tc.tile_pool(name="msgp", bufs=6))
        crepp = ex.enter_context(tc.tile_pool(name="crepp", bufs=2))
        xtp = ex.enter_context(tc.tile_pool(name="xtp", bufs=2))
        accp = ex.enter_context(tc.tile_pool(name="accp", bufs=1))
        ps_gemm = ex.enter_context(tc.tile_pool(name="ps_gemm", bufs=2, space="PSUM"))
        ps_tr = ex.enter_context(tc.tile_pool(name="ps_tr", bufs=2, space="PSUM"))

        nc.gpsimd.load_library(library_config.mlp)

        def load_const(name, src_ap, shape, dtype):
            t = const.tile(shape, dtype, name=name)
            nc.sync.dma_start(t[:], src_ap)
            return t

        w_sb = [load_const(f"w{i}", w_in[i][:], [P, P], BF16) for i in range(3)]
        dinvc = load_const("dinvc", dinv_in[:], [P, NT], F32)
        dinv2c = load_const("dinv2c", dinv2_in[:], [P, NT], F32)
        identb = load_const("identb", ident_in[:], [P, P], BF16)
        idxw = load_const("idxw", idxw_in[:], [P, LT // 16], I16)
        zeros64 = const.tile([HL, P], BF16, name="zeros64")
        nc.gpsimd.memset(zeros64[:], 0.0)

        x_prev = None  # SBUF [P, NT, P] bf16 = a*X (row-major, layers 2,3)
        g_gather = 0   # global gather count: keeps queue_num aligned with
                       # Tile's DMASW lane cycling (mod 8 -> mod 4)

        for layer in range(3):
            crep = crepp.tile([P, NT, P], BF16, name="crep")
            nc.sync.dma_start(
                crep[:], crep_in[:, layer, :].rearrange(
                    "p (t f) -> p t f", t=NT))

            # ---- GEMM: h_sent = (a*X) @ W  (pure-cast epilogue) ----
            h_sent = sb.tile([P, NT, P], BF16, name="h_sent")
            for g in range(0, NT, TG):
                gsz = min(TG, NT - g)
                if layer == 0:
                    xf = sb.tile([P, TG, P], F32, name="xf")
                    nc.sync.dma_start(
                        xf[:, :gsz, :],
                        x_in[g * P:(g + gsz) * P, :].rearrange(
                            "(t p) f -> p t f", p=P))
                    xb = sb.tile([P, TG, P], BF16, name="xb")
                    nc.vector.tensor_copy(xb[:, :gsz, :], xf[:, :gsz, :])
                g_ps = ps_gemm.tile([P, TG, P], F32, space="PSUM", name="g_ps")
                for j in range(gsz):
                    t = g + j
                    xbj = xb[:, j, :] if layer == 0 else x_prev[:, t, :]
                    tr_ps = ps_tr.tile([P, P], BF16, space="PSUM", name="tr_ps")
                    nc.tensor.transpose(out=tr_ps[:], in_=xbj, identity=identb[:])
                    xt = xtp.tile([P, P], BF16, name="xt")
                    nc.vector.tensor_copy(xt[:], tr_ps[:])
                    nc.tensor.matmul(out=g_ps[:, j, :], lhsT=xt[:],
                                     rhs=w_sb[layer][:], start=True, stop=True)
                nc.vector.tensor_copy(h_sent[:, g:g + gsz, :], g_ps[:, :gsz, :])

            # ---- bounce (lane-halves) -> DRAM -> 2 AllGathers ----
            bounce = dram.tile([2 * RS, P], BF16, name="bounce")
            tables = []
            for h in range(2):
                nc.sync.dma_start(
                    bounce[h * RS:h * RS + NT * HL, :].rearrange(
                        "(t p) f -> p t f", p=HL),
                    h_sent[h * HL:(h + 1) * HL, :, :])
                nc.sync.dma_start(
                    bounce[h * RS + NT * HL:(h + 1) * RS, :].rearrange(
                        "(t p) f -> p t f", t=1),
                    zeros64[:].rearrange("p (t f) -> p t f", t=1))
                tbl = dram.tile([TBL, P], BF16, addr_space="Shared",
                                name=f"tbl{h}")
                nc.gpsimd.collective_compute(
                    "AllGather", mybir.AluOpType.bypass,
                    replica_groups=[core_ids],
                    ins=[bounce[h * RS:(h + 1) * RS, :]],
                    outs=[tbl[:]])
                tables.append(tbl)

            # ---- gather calls (4 SWDGE queues round-robin) ----
            msg_tiles = []
            for (hh, start, n) in calls:
                mt = msgp.tile([P, call_size // P, P], BF16, name="mt")
                nc.gpsimd.dma_gather(
                    mt[:, 0:n // P, :], tables[hh][:],
                    idxw[:, start // 16:(start + n) // 16],
                    n, n, P, queue_num=g_gather % 4)
                g_gather += 1
                msg_tiles.append((start, n, mt))

            def call_of(slot):
                for (cs, cn, mt) in msg_tiles:
                    if cs <= slot < cs + cn:
                        return cs, mt
                raise AssertionError

            # ---- segment sum: DVE adds of level slices into acc ----
            acc = accp.tile([P, NT, P], F32, name="acc")
            for (first, t0, ntl, slot) in segs:
                cs, mt = call_of(slot)
                ms = (slot - cs) // P
                if first:
                    nc.vector.tensor_copy(acc[:, t0:t0 + ntl, :],
                                          mt[:, ms:ms + ntl, :])
                else:
                    nc.vector.tensor_tensor(
                        out=acc[:, t0:t0 + ntl, :],
                        in0=acc[:, t0:t0 + ntl, :],
                        in1=mt[:, ms:ms + ntl, :],
                        op=mybir.AluOpType.add)

            # ---- self-loop + bias pre-add + epilogue ----
            for g in range(0, NT, TG):
                gsz = min(TG, NT - g)
                nc.vector.tensor_tensor(
                    out=acc[:, g:g + gsz, :], in0=acc[:, g:g + gsz, :],
                    in1=h_sent[:, g:g + gsz, :], op=mybir.AluOpType.add)
                nc.vector.tensor_tensor(
                    out=acc[:, g:g + gsz, :], in0=acc[:, g:g + gsz, :],
                    in1=crep[:, g:g + gsz, :], op=mybir.AluOpType.add)
            if layer < 2:
                x_prev = sb.tile([P, NT, P], BF16, name="x_next")
                for t in range(NT):
                    nc.scalar.activation(
                        out=x_prev[:, t, :], in_=acc[:, t, :],
                        func=mybir.ActivationFunctionType.Relu,
                        scale=dinv2c[:, t:t + 1])
            else:
                out_sb = sb.tile([P, NT, 64], F32, name="out_sb")
                for t in range(NT):
                    nc.scalar.activation(
                        out=out_sb[:, t, :], in_=acc[:, t, :64],
                        func=mybir.ActivationFunctionType.Copy,
                        scale=dinvc[:, t:t + 1])
                nc.sync.dma_start(
                    out_ext[:, :].rearrange("(t p) f -> p t f", p=P),
                    out_sb[:, :, :])

    nc.compile()
    return nc


def make_in_maps(x, W1, b1, W2, b2, W3, b3, sched, per_core, perm_info):
    """Build per-core input dicts (x permuted and pre-scaled by dinv)."""
    S, NT = sched["S"], sched["NT"]
    n_cores = sched["n_cores"]
    n_nodes = sched["n_nodes"]
    bf = ml_dtypes.bfloat16
    w1 = np.asarray(W1, np.float32).astype(bf)
    w2 = np.asarray(W2, np.float32).astype(bf)
    w3 = np.zeros((P, P), np.float32)
    w3[:, :64] = np.asarray(W3, np.float32)
    w3 = w3.astype(bf)
    identb = np.eye(P, dtype=np.float32).astype(bf)
    dinv = perm_info["dinv"]
    xs = np.asarray(x, np.float32) * dinv[:, None]
    perm_rows = perm_info["perm_rows"]
    xp_all = np.zeros((n_cores * S, P), np.float32)
    xp_all[perm_rows[:n_nodes]] = xs

    bs = [np.asarray(b1, np.float32),
          np.asarray(b2, np.float32),
          np.zeros(P, np.float32)]
    bs[2][:64] = np.asarray(b3, np.float32)

    in_maps = []
    for c in range(n_cores):
        d = per_core[c]
        # crep[d, li, t*P + f] = sqd(node at (c,t,d)) * b_li[f]
        sq = d["sqdc"]  # [P, NT]
        crep = np.zeros((P, 3, NT * P), np.float32)
        for li in range(3):
            crep[:, li, :] = (sq[:, :, None] * bs[li][None, None, :]).reshape(
                P, NT * P)
        in_maps.append({
            "x": xp_all[c * S:(c + 1) * S],
            "w1": w1, "w2": w2, "w3": w3,
            "crep": crep.astype(bf),
            "dinvc": np.ascontiguousarray(d["dinvc"]),
            "dinv2c": np.ascontiguousarray(d["dinv2c"]),
            "identb": identb,
            "idxw": np.ascontiguousarray(d["idxw"]),
        })
    return in_maps


def unshard_output(res_outs, sched, perm_info):
    """Concatenate per-core outputs and un-permute to node order."""
    n_cores = sched["n_cores"]
    n_nodes = sched["n_nodes"]
    full = np.concatenate([np.asarray(res_outs[c]) for c in range(n_cores)],
                          axis=0)
    return full[perm_info["perm_rows"][:n_nodes]]


# ---------------------------------------------------------------------------
N_NODES = 50000
N_CORES = 8
CALL_SIZE = 1024


def _run(inputs, trace=False):
    from concourse.bass_utils import run_bass_kernel_spmd

    x = np.asarray(inputs["x"], np.float32)
    edge_index = np.asarray(inputs["edge_index"])
    sched, per_core, perm_info = preprocess(edge_index, N_NODES, N_CORES,
                                            CALL_SIZE)
    nc = build_nc(sched)
    in_maps = make_in_maps(x, inputs["W1"], inputs["b1"], inputs["W2"],
                           inputs["b2"], inputs["W3"], inputs["b3"],
                           sched, per_core, perm_info)
    res = run_bass_kernel_spmd(nc, in_maps, list(range(N_CORES)), trace=trace)
    out = unshard_output([res.results[c]["out"] for c in range(N_CORES)],
                         sched, perm_info)
    return out.astype(np.float32), res


def kernel(x, edge_index, W1, b1, W2, b2, W3, b3):
    out, _ = _run(dict(x=x, edge_index=edge_index, W1=W1, b1=b1,
                       b2=b2, W2=W2, W3=W3, b3=b3), trace=False)
    return out


# revision 31
# speedup vs baseline: 1.2510x; 1.2510x over previous
"""GCN (3-layer) Bass kernel for 8 TRN2 NeuronCores, SPMD.

Math: out = A_hat @ relu(A_hat @ relu(A_hat @ X W1 + b1) W2 + b2) W3 + b3
A_hat = D^-1/2 (A + I) D^-1/2.

Key design (v2 — DVE-aggregated, lane-slotted gather):
  The PE-throttled bottleneck of v1 (one one-hot matmul per 128 messages)
  is gone: messages are gathered DIRECTLY INTO their destination lane via
  a host-computed permutation, so the segment sum is a plain DVE add of
  [128 lanes, tiles, 128 feat] blocks.  Per layer the PE only does the
  dense GEMM (49 transposes + 49 matmuls).

  - Nodes are permuted host-side: sorted by degree into 49 "bands"; band
    t supplies tile t of every core (128 lanes/core).  Degree-banding
    equalizes per-lane message counts, so the rectangular (lane x level)
    slot grid wastes little.
  - Each node is assigned a half (lane<64 -> table 0) by a greedy that
    balances, for every destination, its in-edges across halves (keeps
    per-(half,lane) slot counts near deg/2).  Table h is AllGathered from
    h_sent[lanes 64h:64h+64, :, :] — a rectangular partition-range DMA.
  - Slot stream per (half): for level k, for tiles t with K[h,t]>k, 128
    slots (lane-major).  Level-k slots for a run of tiles are contiguous,
    so one DVE tensor op accumulates a whole level: acc += mt_slice.
  - Empty slots gather a zeroed pad row of the table.
  - Self loops ride in the stream as ordinary slots (source = own row).
  - Bias enters as acc += crep (crep[d,t,f] = sqrt(deg) * b[f], host
    const), then the ACT epilogue computes relu(dinv^2 * acc) =
    a*relu(a*agg + b) = next layer's pre-scaled x.  Final layer:
    out = dinv * acc (ACT Copy).
"""

from contextlib import ExitStack

import numpy as np
import ml_dtypes

import concourse.bacc as bacc
import concourse.bass as bass
import concourse.mybir as mybir
from concourse.tile import TileContext
from concourse import library_config

BF16 = mybir.dt.bfloat16
F32 = mybir.dt.float32
I16 = mybir.dt.int16
P = 128
HL = 64          # lanes per half
TG = 4           # tiles per PSUM bank group (GEMM)


def preprocess(edge_index, n_nodes, n_cores=8, call_size=4096, seed=0):
    """Host-side graph preprocessing.

    Returns (sched, per_core_data, perm_info).
    perm_info: node -> (core, l) permutation plus inverse for unsharding.
    """
    src = np.asarray(edge_index[0], dtype=np.int64)
    dst = np.asarray(edge_index[1], dtype=np.int64)
    deg = (np.bincount(dst, minlength=n_nodes) + 1).astype(np.float32)
    dinv = (1.0 / np.sqrt(deg)).astype(np.float32)
    sqd = np.sqrt(deg).astype(np.float32)

    NT = (n_nodes + n_cores * P - 1) // (n_cores * P)
    S = NT * P                      # padded nodes per core
    NP = S * n_cores                # padded total
    n_dummy = NP - n_nodes

    # --- node permutation: degree-sorted bands; band t -> tile t ---
    order = np.argsort(-deg, kind="stable")        # real nodes, deg desc
    # node id -> (band, core, slot-in-(core,band))
    core_of = np.empty(NP, dtype=np.int64)
    band_of = np.empty(NP, dtype=np.int64)
    ranked = np.concatenate([order, np.arange(n_nodes, NP)])  # dummies last
    r = np.arange(NP)
    band_of[ranked] = r // (n_cores * P)
    core_of[ranked] = (r % (n_cores * P)) % n_cores

    # --- half balancing (greedy): exactly HL of each (core, band)'s 128
    # nodes to half 0, minimizing per-dst |c0-c1| ---
    # CSR by source over real edges
    e_order = np.argsort(src, kind="stable")
    s_sorted = src[e_order]
    d_sorted = dst[e_order]
    sptr = np.searchsorted(s_sorted, np.arange(n_nodes + 1))
    D = np.zeros(n_nodes, dtype=np.int32)          # c0 - c1 per dst
    quota = np.full((n_cores, NT, 2), HL, dtype=np.int32)
    half_of = np.empty(NP, dtype=np.int8)
    # process high out-degree first (they matter most)
    out_deg = sptr[1:] - sptr[:-1]
    proc = np.argsort(-out_deg, kind="stable")
    for u in proc:
        c, b = core_of[u], band_of[u]
        q0, q1 = quota[c, b, 0], quota[c, b, 1]
        if q0 == 0:
            h = 1
        elif q1 == 0:
            h = 0
        else:
            vs = d_sorted[sptr[u]:sptr[u + 1]]
            s = int(D[vs].sum()) if len(vs) else 0
            h = 1 if s > 0 else 0
        half_of[u] = h
        quota[c, b, h] -= 1
        if sptr[u + 1] > sptr[u]:
            vs = d_sorted[sptr[u]:sptr[u + 1]]
            np.add.at(D, vs, 1 - 2 * h)
    for u in range(n_nodes, NP):                   # dummies fill quotas
        c, b = core_of[u], band_of[u]
        h = 0 if quota[c, b, 0] > 0 else 1
        half_of[u] = h
        quota[c, b, h] -= 1
    assert (quota == 0).all()

    # --- quota-preserving swap refinement (minimize sum D^2 over dsts),
    # with incremental sD maintenance so decisions never go stale ---
    ed_order = np.argsort(d_sorted, kind="stable")   # edges sorted by dst
    in_src = s_sorted[ed_order]
    in_d = d_sorted[ed_order]
    in_ptr = np.searchsorted(in_d, np.arange(n_nodes + 1))
    from scipy.sparse import coo_matrix as _coo
    Adj = _coo((np.ones(len(src), np.int64), (src, dst)),
               shape=(n_nodes, n_nodes)).tocsr()
    sD = np.zeros(NP, np.int64)
    sD[:n_nodes] = Adj @ D.astype(np.int64)
    k_out = np.zeros(NP, np.int64)
    k_out[:n_nodes] = out_deg
    grp_id = core_of * NT + band_of                # node -> group
    grp_members = [np.nonzero(grp_id == g)[0] for g in range(n_cores * NT)]

    def _apply_flip(u, s):
        """Flip node u's half; D at its dsts changes by s (+-2)."""
        if u >= n_nodes:
            return
        vs = d_sorted[sptr[u]:sptr[u + 1]]
        D[vs] += s
        ins = np.concatenate([in_src[in_ptr[v]:in_ptr[v + 1]] for v in vs]) \
            if len(vs) else np.empty(0, np.int64)
        if len(ins):
            np.add.at(sD, ins, s)

    for _ in range(4):
        flipped = 0
        for g in range(n_cores * NT):
            members = grp_members[g]
            m0 = members[half_of[members] == 0]
            m1 = members[half_of[members] == 1]
            gain01 = sD[m0] - k_out[m0]            # flip 0->1 gain (x4)
            gain10 = -sD[m1] - k_out[m1]           # flip 1->0 gain
            c0 = m0[np.argsort(-gain01, kind="stable")]
            c1 = m1[np.argsort(-gain10, kind="stable")]
            g0 = np.sort(gain01)[::-1]
            g1 = np.sort(gain10)[::-1]
            for u0, u1, a, b in zip(c0, c1, g0, g1):
                if a + b <= 0:
                    break
                half_of[u0] = 1
                half_of[u1] = 0
                _apply_flip(u0, -2)
                _apply_flip(u1, 2)
                flipped += 2
        if flipped == 0:
            break

    # --- lane assignment: half0 -> lanes 0..63, half1 -> 64..127 ---
    lane_of = np.empty(NP, dtype=np.int64)
    nodes_by_cb = [[[] for _ in range(NT)] for _ in range(n_cores)]
    for u in ranked:                               # rank order within groups
        nodes_by_cb[core_of[u]][band_of[u]].append(u)
    for c in range(n_cores):
        for b in range(NT):
            grp = nodes_by_cb[c][b]
            assert len(grp) == P
            i0 = i1 = 0
            for u in grp:
                if half_of[u] == 0:
                    lane_of[u] = i0
                    i0 += 1
                else:
                    lane_of[u] = HL + i1
                    i1 += 1
            assert i0 == HL and i1 == HL

    l_of = band_of * P + lane_of                   # local row index
    # inverse permutation for output unshard: out row of node u
    perm_rows = core_of * S + l_of                 # node -> global padded row

    # --- table row index of each node (as source) ---
    RS = NT * HL + HL                              # shard rows per half (+pad)
    ZROW = NT * HL                                 # zero rows at shard tail
    tbl_row = core_of * RS + band_of * HL + (lane_of % HL)

    # --- per-destination source lists (table rows), split by src half ---
    # (self loops are added on-device as a direct DVE add of h_sent)
    msrc = src
    mdst = dst
    mh = half_of[msrc].astype(np.int64)
    mrow = tbl_row[msrc]
    # group by (dst, half): sort by (dst, half, mrow)
    key = (mdst * 2 + mh) * (RS * n_cores + 1) + mrow
    g_order = np.argsort(key, kind="stable")
    gd = mdst[g_order]
    gh = mh[g_order]
    grow = mrow[g_order]
    # counts per (dst, half)
    cnt = np.zeros((NP, 2), dtype=np.int64)
    np.add.at(cnt, (gd, gh), 1)
    # start offset of each (dst, half) run in the sorted stream
    run_len = cnt.reshape(-1)
    run_start = np.concatenate([[0], np.cumsum(run_len)[:-1]])

    # --- K levels per (half, tile): max over cores & lanes, monotone env ---
    K = np.zeros((2, NT), dtype=np.int64)
    cview = cnt.reshape(NP, 2)
    for h in range(2):
        per_node = cview[:, h]
        # per (band) max over all nodes in that band (all cores)
        Kt = np.zeros(NT, dtype=np.int64)
        np.maximum.at(Kt, band_of, per_node)
        K[h] = Kt
    for h in range(2):                             # monotone non-increasing
        for t in range(NT - 2, -1, -1):
            K[h, t] = max(K[h, t], K[h, t + 1])

    # level prefix sizes T[h][k] = #tiles with K[h,t] > k
    T_lvl = [[int((K[h] > k).sum()) for k in range(int(K[h].max()))]
             for h in range(2)]

    # --- build per-core idx streams (k-level-major) ---
    ZIDX = ZROW                                    # zero row of core 0
    node_at = np.empty(NP, dtype=np.int64)         # (c*S + l) -> node id
    node_at[perm_rows] = np.arange(NP)
    L0 = sum(T_lvl[0]) * P
    L1 = sum(T_lvl[1]) * P
    LT = L0 + L1
    per_core = []
    for c in range(n_cores):
        idx_stream = np.full(LT, ZIDX, dtype=np.int64)
        pos = 0
        for h in range(2):
            for k, Tk in enumerate(T_lvl[h]):
                nodes = node_at[c * S + np.arange(Tk * P)]   # (t,d) row-major
                starts = run_start[nodes * 2 + h]
                lens = run_len[nodes * 2 + h]
                sel = lens > k
                idx_stream[pos + np.nonzero(sel)[0]] = grow[starts[sel] + k]
                pos += Tk * P
        assert pos == LT
        idxw = idx_stream.astype(np.int16).reshape(LT // 16, 16).T
        idxw = np.tile(idxw, (8, 1))
        per_core.append(dict(idxw=np.ascontiguousarray(idxw)))

    # --- per-core constants (lane-major) ---
    ids = np.arange(S)
    for c in range(n_cores):
        nods = node_at[c * S + ids]
        dvc = np.zeros((P, NT), dtype=np.float32)
        dv2 = np.zeros((P, NT), dtype=np.float32)
        sq = np.zeros((P, NT), dtype=np.float32)
        real = nods < n_nodes
        dvc[ids[real] % P, ids[real] // P] = dinv[nods[real]]
        dv2[ids[real] % P, ids[real] // P] = dinv[nods[real]] ** 2
        sq[ids[real] % P, ids[real] // P] = sqd[nods[real]]
        per_core[c].update(dinvc=dvc, dinv2c=dv2, sqdc=sq)

    # --- gather calls ---
    calls = []   # (half, start, n)
    for h, (lo, ln) in enumerate(((0, L0), (L0, L1))):
        off = 0
        while off < ln:
            n = min(call_size, ln - off)
            calls.append((h, lo + off, n))
            off += n

    # --- DVE add segments: (is_init, tile0, ntiles, start_slot) ---
    # is_init: level (h=0, k=0) covers every tile and initializes acc.
    segs = []
    pos = 0
    for h in range(2):
        half_lo = 0 if h == 0 else L0
        for k, Tk in enumerate(T_lvl[h]):
            # split [0, Tk) tile range at call boundaries (grid per half)
            t0 = 0
            while t0 < Tk:
                rel = pos + t0 * P - half_lo
                room = (rel // call_size + 1) * call_size - rel
                t1 = min(Tk, t0 + room // P)
                segs.append((h == 0 and k == 0, t0, t1 - t0, pos + t0 * P))
                t0 = t1
            pos += Tk * P
    assert pos == LT
    assert T_lvl[0][0] == NT, "level (0,0) must cover all tiles"

    sched = dict(n_nodes=n_nodes, n_cores=n_cores, S=S, NT=NT, RS=RS,
                 K=K, T_lvl=T_lvl, L0=L0, L1=L1, LT=LT, calls=calls,
                 segs=segs, call_size=call_size)
    perm_info = dict(perm_rows=perm_rows, node_at=node_at, dinv=dinv,
                     sqd=sqd, n_dummy=n_dummy)
    return sched, per_core, perm_info


def build_nc(sched):
    """Build the SPMD Bass graph (identical for all 8 cores).

    v3: table lives in SBUF (SRAM random access — no HBM row-miss cost on
    the gather); gather runs in transpose mode so messages, accumulator
    and x are all FEATURE-major.  Self-loop = W^T @ xT matmul (also
    initializes acc).  No PE transposes except the final output.
    """
    S, NT, RS = sched["S"], sched["NT"], sched["RS"]
    calls, segs = sched["calls"], sched["segs"]
    n_cores = sched["n_cores"]
    call_size = sched["call_size"]
    LT, L0 = sched["LT"], sched["L0"]
    TBL = RS * n_cores
    TTI = TBL // P                   # table tiles in SBUF
    core_ids = list(range(n_cores))

    nc = bacc.Bacc("TRN2", target_bir_lowering=False, num_devices=n_cores,
                   num_swdge_queues=4)

    x_in = nc.dram_tensor("x", [P, S], BF16, kind="ExternalInput")  # xT
    w_in = [nc.dram_tensor(f"w{i+1}", [P, P], BF16, kind="ExternalInput")
            for i in range(3)]
    crep_in = nc.dram_tensor("crep", [P, 3, S], BF16, kind="ExternalInput")
    a2t_in = nc.dram_tensor("a2t", [P, S], BF16, kind="ExternalInput")
    at_in = nc.dram_tensor("at", [P, S], BF16, kind="ExternalInput")
    ident_in = nc.dram_tensor("identb", [P, P], BF16, kind="ExternalInput")
    idxw_in = nc.dram_tensor("idxw", [P, LT // 16], I16, kind="ExternalInput")
    out_ext = nc.dram_tensor("out", [S, 64], F32, kind="ExternalOutput")

    with TileContext(nc) as tc, ExitStack() as ex:
        const = ex.enter_context(tc.tile_pool(name="const", bufs=1))
        dram = ex.enter_context(tc.tile_pool(name="dram", bufs=1, space="DRAM"))
        sb = ex.enter_context(tc.tile_pool(name="sb", bufs=2))
        tblp = ex.enter_context(tc.tile_pool(name="tblp", bufs=1))
        msgp = ex.enter_context(tc.tile_pool(name="msgp", bufs=6))
        crepp = ex.enter_context(tc.tile_pool(name="crepp", bufs=1))
        accp = ex.enter_context(tc.tile_pool(name="accp", bufs=1))
        outp = ex.enter_context(tc.tile_pool(name="outp", bufs=1))
        ps_gemm = ex.enter_context(tc.tile_pool(name="ps_gemm", bufs=2, space="PSUM"))
        ps_self = ex.enter_context(tc.tile_pool(name="ps_self", bufs=2, space="PSUM"))
        ps_tr = ex.enter_context(tc.tile_pool(name="ps_tr", bufs=2, space="PSUM"))

        nc.gpsimd.load_library(library_config.mlp)

        def load_const(name, src_ap, shape, dtype):
            t = const.tile(shape, dtype, name=name)
            nc.sync.dma_start(t[:], src_ap)
            return t

        w_sb = [load_const(f"w{i}", w_in[i][:], [P, P], BF16) for i in range(3)]
        identb = load_const("identb", ident_in[:], [P, P], BF16)
        idxw = load_const("idxw", idxw_in[:], [P, LT // 16], I16)
        zeros64 = const.tile([HL, P], BF16, name="zeros64")
        nc.gpsimd.memset(zeros64[:], 0.0)
        scalep = ex.enter_context(tc.tile_pool(name="scalep", bufs=1))

        x_cur = const.tile([P, S], BF16, name="x0")   # xT, feature-major
        nc.sync.dma_start(x_cur[:], x_in[:])
        g_gather = 0   # keeps queue_num aligned with Tile's DMASW lanes

        for layer in range(3):
            crep = crepp.tile([P, S], BF16, name="crep")
            nc.sync.dma_start(crep[:], crep_in[:, layer, :])

            # ---- GEMM h_sent[node,f] = xT^T @ W ; self/init accT = W^T@xT
            h_sent = sb.tile([P, NT, P], BF16, name="h_sent")
            acc = accp.tile([P, S], F32, name="acc")
            for g in range(0, NT, TG):
                gsz = min(TG, NT - g)
                g_ps = ps_gemm.tile([P, TG, P], F32, space="PSUM", name="g_ps")
                s_ps = ps_self.tile([P, TG, P], F32, space="PSUM", name="s_ps")
                for j in range(gsz):
                    t = g + j
                    xt_t = x_cur[:, t * P:(t + 1) * P]
                    nc.tensor.matmul(out=g_ps[:, j, :], lhsT=xt_t,
                                     rhs=w_sb[layer][:], start=True, stop=True)
                    nc.tensor.matmul(out=s_ps[:, j, :], lhsT=w_sb[layer][:],
                                     rhs=xt_t, start=True, stop=True)
                nc.vector.tensor_copy(h_sent[:, g:g + gsz, :], g_ps[:, :gsz, :])
                # acc starts as self-contribution + bias row
                nc.vector.tensor_tensor(
                    out=acc[:, g * P:(g + gsz) * P],
                    in0=s_ps[:, :gsz, :].rearrange("p t f -> p (t f)"),
                    in1=crep[:, g * P:(g + gsz) * P],
                    op=mybir.AluOpType.add)

            # ---- bounce (lane-halves) -> DRAM -> 2 AllGathers ----
            bounce = dram.tile([2 * RS, P], BF16, name="bounce")
            tbls = []
            for h in range(2):
                nc.sync.dma_start(
                    bounce[h * RS:h * RS + NT * HL, :].rearrange(
                        "(t p) f -> p t f", p=HL),
                    h_sent[h * HL:(h + 1) * HL, :, :])
                nc.sync.dma_start(
                    bounce[h * RS + NT * HL:(h + 1) * RS, :].rearrange(
                        "(t p) f -> p t f", t=1),
                    zeros64[:].rearrange("p (t f) -> p t f", t=1))
                tbl = dram.tile([TBL, P], BF16, addr_space="Shared",
                                name=f"tbl{h}")
                nc.gpsimd.collective_compute(
                    "AllGather", mybir.AluOpType.bypass,
                    replica_groups=[core_ids],
                    ins=[bounce[h * RS:(h + 1) * RS, :]],
                    outs=[tbl[:]])
                tbls.append(tbl)

            # ---- per half: table -> SBUF, SBUF-source transpose gathers,
            #      level adds into feature-major acc ----
            msg_tiles = {}
            seg_i = 0
            for h in range(2):
                stbl = tblp.tile([P, TTI, P], BF16, name="stbl")
                nc.sync.dma_start(
                    stbl[:], tbls[h][:].rearrange("(t p) f -> p t f", p=P))
                for (hh, start, n) in calls:
                    if hh != h:
                        continue
                    mt = msgp.tile([P, 1, call_size], BF16, name="mt")
                    nc.gpsimd.dma_gather(
                        mt[:, :, 0:n], stbl[:].rearrange("p t f -> p (t f)"),
                        idxw[:, start // 16:(start + n) // 16],
                        n, n, P, transpose=True,
                        sbuf_tokens_per_rank=P,
                        sbuf_free_dim_per_rank=P * 2,
                        queue_num=g_gather % 4)
                    g_gather += 1
                    msg_tiles[start] = (start, n, mt)
                # level adds for this half (segs are in slot order)
                while seg_i < len(segs):
                    (_, t0, ntl, slot) = segs[seg_i]
                    if (0 if slot < L0 else 1) != h:
                        break
                    cs = None
                    for s0, (cs0, cn0, mt0) in msg_tiles.items():
                        if cs0 <= slot < cs0 + cn0:
                            cs, mt = cs0, mt0
                    ms = slot - cs
                    nc.vector.tensor_tensor(
                        out=acc[:, t0 * P:(t0 + ntl) * P],
                        in0=acc[:, t0 * P:(t0 + ntl) * P],
                        in1=mt[:, 0, ms:ms + ntl * P],
                        op=mybir.AluOpType.add)
                    seg_i += 1
            assert seg_i == len(segs)

            # ---- epilogue (feature-major) ----
            sc = scalep.tile([P, S], BF16, name="sc")
            nc.sync.dma_start(sc[:], a2t_in[:] if layer < 2 else at_in[:])
            nc.vector.tensor_tensor(
                out=acc[:], in0=acc[:], in1=sc[:], op=mybir.AluOpType.mult)
            if layer < 2:
                xn = sb.tile([P, S], BF16, name="x_next")
                for g in range(0, NT, TG):
                    gsz = min(TG, NT - g)
                    nc.scalar.activation(
                        out=xn[:, g * P:(g + gsz) * P],
                        in_=acc[:, g * P:(g + gsz) * P],
                        func=mybir.ActivationFunctionType.Relu,
                        scale=1.0)
                x_cur = xn
            else:
                out_sb = outp.tile([P, NT, 64], F32, name="out_sb")
                for t in range(NT):
                    accb = outp.tile([P, P], BF16, name="accb")
                    nc.vector.tensor_copy(accb[:], acc[:, t * P:(t + 1) * P])
                    tr_ps = ps_tr.tile([P, P], BF16, space="PSUM", name="tr_ps")
                    nc.tensor.transpose(
                        out=tr_ps[:], in_=accb[:],
                        identity=identb[:])
                    nc.vector.tensor_copy(out_sb[:, t, :], tr_ps[:, :64])
                nc.sync.dma_start(
                    out_ext[:, :].rearrange("(t p) f -> p t f", p=P),
                    out_sb[:, :, :])

    nc.compile()
    return nc


def make_in_maps(x, W1, b1, W2, b2, W3, b3, sched, per_core, perm_info):
    """Build per-core input dicts (x permuted, pre-scaled, TRANSPOSED)."""
    S, NT = sched["S"], sched["NT"]
    n_cores = sched["n_cores"]
    n_nodes = sched["n_nodes"]
    bf = ml_dtypes.bfloat16
    w1 = np.asarray(W1, np.float32).astype(bf)
    w2 = np.asarray(W2, np.float32).astype(bf)
    w3 = np.zeros((P, P), np.float32)
    w3[:, :64] = np.asarray(W3, np.float32)
    w3 = w3.astype(bf)
    identb = np.eye(P, dtype=np.float32).astype(bf)
    dinv = perm_info["dinv"]
    node_at = perm_info["node_at"]
    xs = np.asarray(x, np.float32) * dinv[:, None]
    perm_rows = perm_info["perm_rows"]
    xp_all = np.zeros((n_cores * S, P), np.float32)
    xp_all[perm_rows[:n_nodes]] = xs

    bs = [np.asarray(b1, np.float32),
          np.asarray(b2, np.float32),
          np.zeros(P, np.float32)]
    bs[2][:64] = np.asarray(b3, np.float32)

    dinv_l = np.zeros(n_cores * S, np.float32)   # per padded row
    sqd_l = np.zeros(n_cores * S, np.float32)
    real = node_at < n_nodes
    dinv_l[real] = dinv[node_at[real]]
    sqd_l[real] = perm_info["sqd"][node_at[real]]

    in_maps = []
    for c in range(n_cores):
        d = per_core[c]
        sq = sqd_l[c * S:(c + 1) * S]            # [S]
        dv = dinv_l[c * S:(c + 1) * S]
        crep = np.empty((P, 3, S), np.float32)
        for li in range(3):
            crep[:, li, :] = bs[li][:, None] * sq[None, :]
        in_maps.append({
            "x": np.ascontiguousarray(xp_all[c * S:(c + 1) * S].T).astype(bf),
            "w1": w1, "w2": w2, "w3": w3,
            "crep": crep.astype(bf),
            "a2t": np.broadcast_to(dv * dv, (P, S)).astype(bf),
            "at": np.broadcast_to(dv, (P, S)).astype(bf),
            "identb": identb,
            "idxw": np.ascontiguousarray(d["idxw"]),
        })
    return in_maps


def unshard_output(res_outs, sched, perm_info):
    """Concatenate per-core outputs and un-permute to node order."""
    n_cores = sched["n_cores"]
    n_nodes = sched["n_nodes"]
    full = np.concatenate([np.asarray(res_outs[c]) for c in range(n_cores)],
                          axis=0)
    return full[perm_info["perm_rows"][:n_nodes]]


# ---------------------------------------------------------------------------
N_NODES = 50000
N_CORES = 8
CALL_SIZE = 512


def _run(inputs, trace=False):
    from concourse.bass_utils import run_bass_kernel_spmd

    x = np.asarray(inputs["x"], np.float32)
    edge_index = np.asarray(inputs["edge_index"])
    sched, per_core, perm_info = preprocess(edge_index, N_NODES, N_CORES,
                                            CALL_SIZE)
    nc = build_nc(sched)
    in_maps = make_in_maps(x, inputs["W1"], inputs["b1"], inputs["W2"],
                           inputs["b2"], inputs["W3"], inputs["b3"],
                           sched, per_core, perm_info)
    res = run_bass_kernel_spmd(nc, in_maps, list(range(N_CORES)), trace=trace)
    out = unshard_output([res.results[c]["out"] for c in range(N_CORES)],
                         sched, perm_info)
    return out.astype(np.float32), res


def kernel(x, edge_index, W1, b1, W2, b2, W3, b3):
    out, _ = _run(dict(x=x, edge_index=edge_index, W1=W1, b1=b1,
                       b2=b2, W2=W2, W3=W3, b3=b3), trace=False)
    return out
